# revision 1
# baseline (speedup 1.0000x reference)
"""Trainium2 Bass kernel for nn_LinformerProjectionEntireOutImg.

Math: the reference's softmax is over a constant tensor -> uniform 1/64, so
the whole net collapses to a linear pipeline:
  T[b,q,i,j]  = sum_p cp[b, p*128+q, i, :] @ wc[p*128+q, :, j]   (q = n mod 128)
  S[b, r]     = T.reshape(B, 8192),  r = q*64 + i*8 + j
  P2[b,e]     = S @ E_proj.reshape(8192, 256)
  v[b,k]      = (P2[b,k]+P2[b,64+k]+P2[b,128+k]+P2[b,192+k])/64 + rel[k]
  out[b,o,i,j]= sum_m v[b, i*8+m] * w_next[o, m, j]
Sharding: core c owns capsule groups q in [16c, 16c+16) (== heads 4c..4c+4),
batch unsharded. Each core reads a disjoint 1/8 of current_pose/w_current and
1/8 of E_proj. The pipeline is linear, so each core emits its partial output
(core 0 carries the rel_embedd affine term) and the unshard is a sum.
The 1/64 is folded into E on the host; stage 1/2 run in bf16 (fp32 PSUM
accumulate), stage 3 in fp32.
"""

import os

import numpy as np

_STATE: dict = {}

B, OUT_N, POSE = 32, 64, 64
NCORES = 8

# p-chunk boundaries for the streamed stage-1 operand: small first chunks so
# the first matmul starts early, then large chunks to amortize DMA issue.
P_BOUNDS = [0, 2, 4, 10, 16, 22, 28, 34, 40, 46, 52, 58, 64]


def _build_nc():
    import concourse.mybir as mybir
    from concourse import bacc
    from concourse.tile import TileContext

    f32 = mybir.dt.float32
    bf16 = mybir.dt.bfloat16
    nc = bacc.Bacc()
    AW = nc.dram_tensor("aw_pack", [128, 64 * 384], bf16, kind="ExternalInput")
    E = nc.dram_tensor("e_pack", [128, 2048], bf16, kind="ExternalInput")
    REL = nc.dram_tensor("rel32", [32, 64], f32, kind="ExternalInput")
    WN = nc.dram_tensor("wn_pack", [8, 512], f32, kind="ExternalInput")
    IDT = nc.dram_tensor("ident32", [32, 32], f32, kind="ExternalInput")
    OUT = nc.dram_tensor("out", [2, 128, 512], f32, kind="ExternalOutput")

    with TileContext(nc) as tc:
        with (
            tc.tile_pool(name="apool", bufs=len(P_BOUNDS) - 1) as apool,
            tc.tile_pool(name="epool", bufs=1) as epool,
            tc.tile_pool(name="cpool", bufs=1) as cpool,
            tc.tile_pool(name="spool", bufs=1) as spool,
            tc.tile_pool(name="pp", bufs=1, space="PSUM") as pp,
            tc.tile_pool(name="pp3", bufs=2, space="PSUM") as pp3,
        ):
            # AW chunk DMAs, alternating between the two HWDGE queues.
            awts = []
            et = None
            relt = idt = wnt = None
            for ci in range(len(P_BOUNDS) - 1):
                w = (P_BOUNDS[ci + 1] - P_BOUNDS[ci]) * 384
                awt = apool.tile([128, w], bf16, tag="aw")
                eng = (nc.sync, nc.scalar)[ci % 2]
                eng.dma_start(
                    out=awt[:],
                    in_=AW[:, P_BOUNDS[ci] * 384 : P_BOUNDS[ci + 1] * 384],
                )
                awts.append(awt)
                if ci == 1:
                    et = epool.tile([128, 2048], bf16, tag="e")
                    nc.scalar.dma_start(out=et[:], in_=E[:])
                    relt = cpool.tile([32, 64], f32, tag="rel")
                    nc.sync.dma_start(out=relt[:], in_=REL[:])
                    idt = cpool.tile([32, 32], f32, tag="idt")
                    nc.sync.dma_start(out=idt[:], in_=IDT[:])
                    wnt = cpool.tile([8, 512], f32, tag="wn")
                    nc.sync.dma_start(out=wnt[:], in_=WN[:])

            # stage 1: T[(q,j),(i,b)] = sum_p Wblk_p.T @ A_p  (block-diag over q)
            # Two interleaved accumulation chains (even/odd p) in separate
            # PSUM banks so per-matmul ordering waits don't serialize the PE.
            o_ps0 = pp.tile([128, 256], f32, tag="o_ps0")
            o_ps1 = pp.tile([128, 256], f32, tag="o_ps1")
            for ci in range(len(P_BOUNDS) - 1):
                awt = awts[ci]
                for t in range(P_BOUNDS[ci + 1] - P_BOUNDS[ci]):
                    p = P_BOUNDS[ci] + t
                    tgt = o_ps0 if p % 2 == 0 else o_ps1
                    nc.tensor.matmul(
                        tgt[:],
                        awt[:, t * 384 + 256 : (t + 1) * 384],
                        awt[:, t * 384 : t * 384 + 256],
                        start=(p < 2),
                        stop=(p >= 62),
                    )
            o_half = spool.tile([128, 256], f32, tag="ohalf")
            nc.vector.tensor_copy(o_half[:], o_ps0[:])
            o_sb = spool.tile([128, 256], bf16, tag="osb")
            nc.vector.tensor_add(o_sb[:], o_half[:], o_ps1[:])

            # stage 2: P2[b,e] += O[:, i-cols].T @ (E_i/64)  (accumulate over i)
            p2_ps = pp.tile([32, 256], f32, tag="p2_ps")
            for i in range(8):
                nc.tensor.matmul(
                    p2_ps[:],
                    o_sb[:, i * 32 : (i + 1) * 32],
                    et[:, i * 256 : (i + 1) * 256],
                    start=(i == 0),
                    stop=(i == 7),
                )

            # fold 256 -> 64 and add rel (rel is zeros on cores 1..7)
            p2_sb = spool.tile([32, 256], f32, tag="p2sb")
            nc.vector.tensor_copy(p2_sb[:], p2_ps[:])
            v1 = spool.tile([32, 64], f32, tag="v1")
            nc.vector.tensor_add(v1[:], p2_sb[:, 0:64], p2_sb[:, 64:128])
            v2 = spool.tile([32, 64], f32, tag="v2")
            nc.vector.tensor_add(v2[:], p2_sb[:, 128:192], p2_sb[:, 192:256])
            nc.vector.tensor_add(v1[:], v1[:], v2[:])
            vs = spool.tile([32, 64], f32, tag="vs")
            nc.vector.tensor_add(vs[:], v1[:], relt[:])

            # transpose v slices: vt[m, i*32+b] = v[b, i*8+m] (partition base 0)
            vt_ps = pp.tile([8, 256], f32, tag="vt_ps")
            for i in range(8):
                nc.tensor.transpose(
                    vt_ps[:, i * 32 : (i + 1) * 32],
                    vs[:, i * 8 : (i + 1) * 8],
                    idt[:],
                )
            vt_sb = spool.tile([8, 256], f32, tag="vt")
            nc.vector.tensor_copy(vt_sb[:], vt_ps[:])

            # stage 3: out_h[(i4,b),(o,j)] = vt[:, h-cols].T @ wn[m,(o,j)]
            for h in range(2):
                o3 = pp3.tile([128, 512], f32, tag="o3")
                nc.tensor.matmul(
                    o3[:],
                    vt_sb[:, h * 128 : (h + 1) * 128],
                    wnt[:],
                    start=True,
                    stop=True,
                )
                o3_sb = spool.tile([128, 512], f32, tag="o3sb")
                nc.vector.tensor_copy(o3_sb[:], o3[:])
                nc.sync.dma_start(out=OUT[h], in_=o3_sb[:])
    nc.finalize()
    return nc


def _prepack(current_pose, w_current, w_next, E_proj, rel_embedd):
    import ml_dtypes

    cp = np.ascontiguousarray(current_pose, dtype=np.float32)
    wc = np.ascontiguousarray(w_current, dtype=np.float32).reshape(64, 8, 16, 8, 8)
    # A_all[c, p, (q,m), (i,b)]
    cp6 = cp.reshape(B, 64, 8, 16, 8, 8)  # (b, p, c, q, i, m)
    a_all = np.ascontiguousarray(cp6.transpose(2, 1, 3, 5, 4, 0)).reshape(
        8, 64, 128, 256
    )
    # Wblk_all[c, p, (q,m), (q',j)] block-diagonal
    w_all = np.zeros((8, 64, 16, 8, 16, 8), dtype=np.float32)
    wc_t = wc.transpose(1, 0, 2, 3, 4)  # (c, p, q, m, j)
    for q in range(16):
        w_all[:, :, q, :, q, :] = wc_t[:, :, q]
    w_all = w_all.reshape(8, 64, 128, 128)
    aw_all = np.concatenate([a_all, w_all], axis=-1)  # (c, p, 128, 384)
    # -> (c, part, (p, x)) flat columns, bf16
    aw_all = np.ascontiguousarray(
        aw_all.transpose(0, 2, 1, 3), dtype=ml_dtypes.bfloat16
    ).reshape(8, 128, 64 * 384)
    # E4[c, i, (q,j), e] with the 1/64 fold baked in
    e5 = (np.asarray(E_proj, dtype=np.float32) / 64.0).reshape(8, 16, 8, 8, 256)
    e_all = np.ascontiguousarray(e5.transpose(0, 2, 1, 3, 4)).reshape(8, 8, 128, 256)
    e_all = np.ascontiguousarray(
        e_all.transpose(0, 2, 1, 3), dtype=ml_dtypes.bfloat16
    ).reshape(8, 128, 2048)
    # rel tile: only core 0 carries the affine term
    rel_all = np.zeros((8, 32, 64), dtype=np.float32)
    rel_all[0] = np.broadcast_to(
        np.asarray(rel_embedd, dtype=np.float32).reshape(1, 64), (32, 64)
    )
    wn_pack = np.ascontiguousarray(
        np.asarray(w_next, dtype=np.float32).transpose(1, 0, 2).reshape(8, 512)
    )
    ident = np.eye(32, dtype=np.float32)
    in_maps = []
    for c in range(NCORES):
        in_maps.append(
            {
                "aw_pack": aw_all[c],
                "e_pack": e_all[c],
                "rel32": rel_all[c],
                "wn_pack": wn_pack,
                "ident32": ident,
            }
        )
    return in_maps


def kernel(current_pose, w_current, w_next, E_proj, rel_embedd):
    from concourse import bass_utils

    if "nc" not in _STATE:
        _STATE["nc"] = _build_nc()
    nc = _STATE["nc"]
    in_maps = _prepack(current_pose, w_current, w_next, E_proj, rel_embedd)
    trace = os.environ.get("KERNEL_TRACE") == "1"
    res = bass_utils.run_bass_kernel_spmd(
        nc, in_maps, core_ids=list(range(NCORES)), trace=trace
    )
    _STATE["last_result"] = res
    acc = np.zeros((2, 128, 512), dtype=np.float32)
    for c in range(NCORES):
        acc += res.results[c]["out"]
    # [h, (i4, b), (o, j)] -> (b, o, h*4+i4, j)
    out = (
        acc.reshape(2, 4, 32, 64, 8)
        .transpose(2, 3, 0, 1, 4)
        .reshape(B, OUT_N, POSE)
    )
    return np.ascontiguousarray(out[:, None, :, :])



# revision 6
# speedup vs baseline: 1.4668x; 1.4668x over previous
"""Trainium2 Bass kernel for nn_LinformerProjectionEntireOutImg.

Math: the reference's softmax is over a constant tensor -> uniform 1/64, so
the whole net collapses to a linear pipeline:
  T[b,q,i,j]  = sum_p cp[b, p*128+q, i, :] @ wc[p*128+q, :, j]   (q = n mod 128)
  P2[b,e]     = sum_{q,i,j} T[b,q,i,j] * Ered[(q,i,j), e]
  out[b,o,i,j]= sum_m (P2[b, i*8+m] + rel[i*8+m]) * w_next[o, m, j]
where Ered folds the uniform pooling: Ered[nh,s,e] = sum_k E[nh,s,e+64k]/64.

Sharding: core c owns capsule groups q in [16c, 16c+16) (== heads 4c..4c+4),
batch unsharded. Each core reads a disjoint 1/8 of current_pose/w_current and
its 4 heads of the folded E. The pipeline is linear, so each core emits its
partial P2 (32x64 f32, 8 KB) and the unshard is a sum; the tiny affine
stage-3 epilogue (rel add + 8x8 w_next matmul, ~2 MFLOP) runs on host.

Device stage 1: q's are processed in groups g of 4 (PE matmul output base
partition must be a multiple of 32). Per (g, pc8-step) one matmul
  o_ps[32g + u*8+j, (i,b)] += Wblk[(u',p4,m), (u,j)].T @ A[(u',p4,m), (i,b)]
with Wblk block-diagonal over u==u' (4x zero pad on W only: 512 KB vs the
2.1 MB a 16-wide block-diagonal would cost). 16-step accumulation chains per
group, two groups interleaved so consecutive matmuls hit different PSUM rows.
A streams as 16 contiguous 256 KB chunks round-robin across the two HWDGE
queues (sync/scalar). Stage 2 contracts (q,j) x i against the folded E in two
64-partition halves so the first half overlaps the tail of the A stream.
All device compute is bf16 with f32 PSUM accumulation.
"""

import os

import numpy as np

_STATE: dict = {}

B, OUT_N, POSE = 32, 64, 64
NCORES = 8
NQ = 16  # capsule groups per core; 4 PE groups of 4

def _build_nc():
    import concourse.mybir as mybir
    from concourse import bacc
    from concourse.tile import TileContext

    f32 = mybir.dt.float32
    bf16 = mybir.dt.bfloat16
    nc = bacc.Bacc()
    # a_pack chunk k = g*4+qt: [128=(u',p4,m), 1024=(pc8l, i, b)]
    A = nc.dram_tensor("a_pack", [NQ, 128, 1024], bf16, kind="ExternalInput")
    # w_pack: [128=(u',p4,m), 2048=(g, pc8, u, j)], block-diag over u==u'
    W = nc.dram_tensor("w_pack", [128, 2048], bf16, kind="ExternalInput")
    # e_pack: [128=(q,j), 512=(i,e)], pool-folded, 1/64 baked in
    E = nc.dram_tensor("e_pack", [128, 512], bf16, kind="ExternalInput")
    OUT = nc.dram_tensor("out", [32, 64], f32, kind="ExternalOutput")

    with TileContext(nc) as tc:
        with (
            tc.tile_pool(name="apool", bufs=NQ) as apool,
            tc.tile_pool(name="cpool", bufs=1) as cpool,
            tc.tile_pool(name="spool", bufs=1) as spool,
            tc.tile_pool(name="pp1", bufs=1, space="PSUM") as pp1,
            tc.tile_pool(name="pp2", bufs=1, space="PSUM") as pp2,
        ):
            # W first on sync (needed by the first chains); E second on
            # scalar (needed by stage-2 half 0, well before the A tail).
            # A chunks alternate g-pair-wise: sync carries g0/g2, scalar
            # g1/g3, interleaved by quarter so arrival matches PE order.
            wt = cpool.tile([128, 2048], bf16, tag="w")
            nc.sync.dma_start(out=wt[:], in_=W[:])
            et = cpool.tile([128, 512], bf16, tag="e")
            ats = [None] * NQ
            for gp in range(2):  # g-pairs (0,1) then (2,3)
                for qt in range(4):
                    for u in range(2):
                        g = gp * 2 + u
                        k = g * 4 + qt
                        at = apool.tile([128, 1024], bf16, tag="a")
                        (nc.sync, nc.scalar)[u].dma_start(out=at[:], in_=A[k])
                        ats[k] = at
                    if gp == 0 and qt == 0:
                        nc.scalar.dma_start(out=et[:], in_=E[:])

            # PE out base partitions are limited to {0,32,64}; use one
            # [64,256] PSUM tile per g-pair so each group lands at base 0/32.
            o_psA = pp1.tile([64, 256], f32, tag="o_psA")
            o_psB = pp1.tile([64, 256], f32, tag="o_psB")
            o_ps = (o_psA, o_psB)
            o_sb = spool.tile([128, 256], bf16, tag="o_sb")
            p2 = pp2.tile([32, 64], f32, tag="p2")

            def stage2_half(h):
                nc.vector.tensor_copy(
                    o_sb[h * 64 : (h + 1) * 64, :], o_ps[h][:]
                )
                for i in range(8):
                    nc.tensor.matmul(
                        p2[:],
                        o_sb[h * 64 : (h + 1) * 64, i * 32 : (i + 1) * 32],
                        et[h * 64 : (h + 1) * 64, i * 64 : (i + 1) * 64],
                        start=(h == 0 and i == 0),
                        stop=(h == 1 and i == 7),
                    )

            for gp in range(2):
                for s in range(16):  # pc8 step
                    qt, pc8l = s // 4, s % 4
                    for u in range(2):
                        g = gp * 2 + u
                        nc.tensor.matmul(
                            o_ps[gp][u * 32 : (u + 1) * 32, :],
                            wt[:, g * 512 + s * 32 : g * 512 + s * 32 + 32],
                            ats[g * 4 + qt][:, pc8l * 256 : (pc8l + 1) * 256],
                            start=(s == 0),
                            stop=(s == 15),
                        )
                stage2_half(gp)

            v_sb = spool.tile([32, 64], f32, tag="v")
            nc.vector.tensor_copy(v_sb[:], p2[:])
            nc.sync.dma_start(out=OUT[:], in_=v_sb[:])
    nc.finalize()
    return nc


def _prepack(current_pose, w_current, E_proj):
    import ml_dtypes

    bf16 = ml_dtypes.bfloat16
    # A[c, g*4+qt, (u',p4,m), (pc8l,i,b)]
    #   = cp[b, ((qt*4+pc8l)*4+p4)*128 + 16c + 4g + u', i*8+m]
    cp = np.ascontiguousarray(current_pose, dtype=np.float32)
    a9 = cp.reshape(B, 4, 4, 4, 8, 4, 4, 8, 8)  # (b,qt,pc8l,p4,c,g,u',i,m)
    a_all = np.ascontiguousarray(
        a9.transpose(4, 5, 1, 6, 3, 8, 2, 7, 0), dtype=bf16
    ).reshape(NCORES, NQ, 128, 1024)
    # W[c, (u',p4,m), (g,pc8,u,j)] block-diag over u==u'
    wc = np.asarray(w_current, dtype=np.float32).reshape(16, 4, 8, 4, 4, 8, 8)
    # (pc8, p4, c, g, u, m, j)
    w_all = np.zeros((NCORES, 4, 4, 8, 4, 16, 4, 8), dtype=np.float32)
    for u in range(4):
        # (c, p4, m, g, pc8, j)
        w_all[:, u, :, :, :, :, u, :] = wc[:, :, :, :, u, :, :].transpose(
            2, 1, 4, 3, 0, 5
        )
    w_all = np.ascontiguousarray(w_all, dtype=bf16).reshape(NCORES, 128, 2048)
    # E[c, (q,j), (i,e)] from the pool-folded projection (1/64 baked in)
    er = np.asarray(E_proj, dtype=np.float32).reshape(32, 256, 4, 64).sum(axis=2)
    er = (er / 64.0).reshape(8, 4, 4, 8, 8, 64)  # (c, nh_loc, s_hi, i, j, e)
    e_all = np.ascontiguousarray(
        er.transpose(0, 1, 2, 4, 3, 5), dtype=bf16
    ).reshape(NCORES, 128, 512)
    return [
        {"a_pack": a_all[c], "w_pack": w_all[c], "e_pack": e_all[c]}
        for c in range(NCORES)
    ]


def kernel(current_pose, w_current, w_next, E_proj, rel_embedd):
    from concourse import bass_utils

    if "nc" not in _STATE:
        _STATE["nc"] = _build_nc()
    nc = _STATE["nc"]
    in_maps = _prepack(current_pose, w_current, E_proj)
    trace = os.environ.get("KERNEL_TRACE") == "1"
    res = bass_utils.run_bass_kernel_spmd(
        nc, in_maps, core_ids=list(range(NCORES)), trace=trace
    )
    _STATE["last_result"] = res
    v = np.zeros((B, POSE), dtype=np.float32)
    for c in range(NCORES):
        v += res.results[c]["out"]
    # host epilogue (~2 MFLOP): rel add + next-layer 8x8 pose matmul
    npc = v + np.asarray(rel_embedd, dtype=np.float32).reshape(1, POSE)
    wn = np.asarray(w_next, dtype=np.float32)  # (OUT_N, 8, 8)
    out = np.einsum("bim,omj->boij", npc.reshape(B, 8, 8), wn, optimize=True)
    return np.ascontiguousarray(out.reshape(B, 1, OUT_N, POSE), dtype=np.float32)


# revision 7
# speedup vs baseline: 1.4712x; 1.0030x over previous
"""Trainium2 Bass kernel for nn_LinformerProjectionEntireOutImg.

Math: the reference's softmax is over a constant tensor -> uniform 1/64, so
the whole net collapses to a linear pipeline:
  T[b,q,i,j]  = sum_p cp[b, p*128+q, i, :] @ wc[p*128+q, :, j]   (q = n mod 128)
  P2[b,e]     = sum_{q,i,j} T[b,q,i,j] * Ered[(q,i,j), e]
  out[b,o,i,j]= sum_m (P2[b, i*8+m] + rel[i*8+m]) * w_next[o, m, j]
where Ered folds the uniform pooling: Ered[nh,s,e] = sum_k E[nh,s,e+64k]/64.

Sharding: core c owns capsule groups q in [16c, 16c+16) (== heads 4c..4c+4),
batch unsharded. Each core reads a disjoint 1/8 of current_pose/w_current and
its 4 heads of the folded E. The pipeline is linear, so each core emits its
partial P2 (32x64 f32, 8 KB) and the unshard is a sum; the tiny affine
stage-3 epilogue (rel add + 8x8 w_next matmul, ~2 MFLOP) runs on host.

Device stage 1: q's are processed in groups g of 4 (PE matmul output base
partition must be a multiple of 32). Per (g, pc8-step) one matmul
  o_ps[32g + u*8+j, (i,b)] += Wblk[(u',p4,m), (u,j)].T @ A[(u',p4,m), (i,b)]
with Wblk block-diagonal over u==u' (4x zero pad on W only: 512 KB vs the
2.1 MB a 16-wide block-diagonal would cost). 16-step accumulation chains per
group, two groups interleaved so consecutive matmuls hit different PSUM rows.
A streams as 16 contiguous 256 KB chunks round-robin across the two HWDGE
queues (sync/scalar). Stage 2 contracts (q,j) x i against the folded E in two
64-partition halves so the first half overlaps the tail of the A stream.
All device compute is bf16 with f32 PSUM accumulation.
"""

import os

import numpy as np

_STATE: dict = {}

B, OUT_N, POSE = 32, 64, 64
NCORES = 8
NQ = 16  # capsule groups per core; 4 PE groups of 4

def _build_nc():
    import concourse.mybir as mybir
    from concourse import bacc
    from concourse.tile import TileContext

    f32 = mybir.dt.float32
    bf16 = mybir.dt.bfloat16
    nc = bacc.Bacc()
    # a_pack chunk k = g*4+qt: [128=(u',p4,m), 1024=(pc8l, i, b)]
    A = nc.dram_tensor("a_pack", [NQ, 128, 1024], bf16, kind="ExternalInput")
    # w_pack: [128=(u',p4,m), 2048=(g, pc8, u, j)], block-diag over u==u'
    W = nc.dram_tensor("w_pack", [128, 2048], bf16, kind="ExternalInput")
    # e_pack: [128=(q,j), 512=(i,e)], pool-folded, 1/64 baked in
    E = nc.dram_tensor("e_pack", [128, 512], bf16, kind="ExternalInput")
    OUT = nc.dram_tensor("out", [32, 64], f32, kind="ExternalOutput")

    with TileContext(nc) as tc:
        with (
            tc.tile_pool(name="apool", bufs=NQ) as apool,
            tc.tile_pool(name="cpool", bufs=1) as cpool,
            tc.tile_pool(name="spool", bufs=1) as spool,
            tc.tile_pool(name="pp1", bufs=1, space="PSUM") as pp1,
            tc.tile_pool(name="pp2", bufs=1, space="PSUM") as pp2,
        ):
            # Per-g W pieces are issued just before that g's A chunks so no
            # queue is front-loaded; sync carries g0/g2 (+out), scalar
            # g1/g3 (+E), interleaved by quarter so arrival matches PE
            # consumption order. E rides early on scalar (needed by stage-2
            # half 0, well before the A tail).
            wt = cpool.tile([128, 2048], bf16, tag="w")
            et = cpool.tile([128, 512], bf16, tag="e")
            ats = [None] * NQ
            for gp in range(2):  # g-pairs (0,1) then (2,3)
                for u in range(2):
                    g = gp * 2 + u
                    eng = (nc.sync, nc.scalar)[u]
                    eng.dma_start(
                        out=wt[:, g * 512 : (g + 1) * 512],
                        in_=W[:, g * 512 : (g + 1) * 512],
                    )
                for qt in range(4):
                    for u in range(2):
                        g = gp * 2 + u
                        k = g * 4 + qt
                        at = apool.tile([128, 1024], bf16, tag="a")
                        (nc.sync, nc.scalar)[u].dma_start(out=at[:], in_=A[k])
                        ats[k] = at
                    if gp == 0 and qt == 0:
                        nc.scalar.dma_start(out=et[:], in_=E[:])

            # PE out base partitions are limited to {0,32,64}; use one
            # [64,256] PSUM tile per g-pair so each group lands at base 0/32.
            o_psA = pp1.tile([64, 256], f32, tag="o_psA")
            o_psB = pp1.tile([64, 256], f32, tag="o_psB")
            o_ps = (o_psA, o_psB)
            o_sb = spool.tile([128, 256], bf16, tag="o_sb")
            p2 = pp2.tile([32, 64], f32, tag="p2")

            def stage2_half(h):
                nc.vector.tensor_copy(
                    o_sb[h * 64 : (h + 1) * 64, :], o_ps[h][:]
                )
                for i in range(8):
                    nc.tensor.matmul(
                        p2[:],
                        o_sb[h * 64 : (h + 1) * 64, i * 32 : (i + 1) * 32],
                        et[h * 64 : (h + 1) * 64, i * 64 : (i + 1) * 64],
                        start=(h == 0 and i == 0),
                        stop=(h == 1 and i == 7),
                    )

            for gp in range(2):
                for s in range(16):  # pc8 step
                    qt, pc8l = s // 4, s % 4
                    for u in range(2):
                        g = gp * 2 + u
                        nc.tensor.matmul(
                            o_ps[gp][u * 32 : (u + 1) * 32, :],
                            wt[:, g * 512 + s * 32 : g * 512 + s * 32 + 32],
                            ats[g * 4 + qt][:, pc8l * 256 : (pc8l + 1) * 256],
                            start=(s == 0),
                            stop=(s == 15),
                        )
                stage2_half(gp)

            v_sb = spool.tile([32, 64], f32, tag="v")
            nc.vector.tensor_copy(v_sb[:], p2[:])
            nc.sync.dma_start(out=OUT[:], in_=v_sb[:])
    nc.finalize()
    return nc


def _prepack(current_pose, w_current, E_proj):
    import ml_dtypes

    bf16 = ml_dtypes.bfloat16
    # A[c, g*4+qt, (u',p4,m), (pc8l,i,b)]
    #   = cp[b, ((qt*4+pc8l)*4+p4)*128 + 16c + 4g + u', i*8+m]
    cp = np.ascontiguousarray(current_pose, dtype=np.float32)
    a9 = cp.reshape(B, 4, 4, 4, 8, 4, 4, 8, 8)  # (b,qt,pc8l,p4,c,g,u',i,m)
    a_all = np.ascontiguousarray(
        a9.transpose(4, 5, 1, 6, 3, 8, 2, 7, 0), dtype=bf16
    ).reshape(NCORES, NQ, 128, 1024)
    # W[c, (u',p4,m), (g,pc8,u,j)] block-diag over u==u'
    wc = np.asarray(w_current, dtype=np.float32).reshape(16, 4, 8, 4, 4, 8, 8)
    # (pc8, p4, c, g, u, m, j)
    w_all = np.zeros((NCORES, 4, 4, 8, 4, 16, 4, 8), dtype=np.float32)
    for u in range(4):
        # (c, p4, m, g, pc8, j)
        w_all[:, u, :, :, :, :, u, :] = wc[:, :, :, :, u, :, :].transpose(
            2, 1, 4, 3, 0, 5
        )
    w_all = np.ascontiguousarray(w_all, dtype=bf16).reshape(NCORES, 128, 2048)
    # E[c, (q,j), (i,e)] from the pool-folded projection (1/64 baked in)
    er = np.asarray(E_proj, dtype=np.float32).reshape(32, 256, 4, 64).sum(axis=2)
    er = (er / 64.0).reshape(8, 4, 4, 8, 8, 64)  # (c, nh_loc, s_hi, i, j, e)
    e_all = np.ascontiguousarray(
        er.transpose(0, 1, 2, 4, 3, 5), dtype=bf16
    ).reshape(NCORES, 128, 512)
    return [
        {"a_pack": a_all[c], "w_pack": w_all[c], "e_pack": e_all[c]}
        for c in range(NCORES)
    ]


def kernel(current_pose, w_current, w_next, E_proj, rel_embedd):
    from concourse import bass_utils

    if "nc" not in _STATE:
        _STATE["nc"] = _build_nc()
    nc = _STATE["nc"]
    in_maps = _prepack(current_pose, w_current, E_proj)
    trace = os.environ.get("KERNEL_TRACE") == "1"
    res = bass_utils.run_bass_kernel_spmd(
        nc, in_maps, core_ids=list(range(NCORES)), trace=trace
    )
    _STATE["last_result"] = res
    v = np.zeros((B, POSE), dtype=np.float32)
    for c in range(NCORES):
        v += res.results[c]["out"]
    # host epilogue (~2 MFLOP): rel add + next-layer 8x8 pose matmul
    npc = v + np.asarray(rel_embedd, dtype=np.float32).reshape(1, POSE)
    wn = np.asarray(w_next, dtype=np.float32)  # (OUT_N, 8, 8)
    out = np.einsum("bim,omj->boij", npc.reshape(B, 8, 8), wn, optimize=True)
    return np.ascontiguousarray(out.reshape(B, 1, OUT_N, POSE), dtype=np.float32)
